# revision 1
# baseline (speedup 1.0000x reference)
"""CBOW negative-sampling loss kernel for 8 Trainium2 NeuronCores.

The reference computes one-hot @ table matmuls (embedding lookups in
disguise) followed by a tiny log-sigmoid loss.  Device-side algorithm:

Phase A (index extraction, streaming):
  Every one-hot row (50000 wide) is laid out as 4 partitions x 12500.
  Stream chunks, multiply by an iota tile whose value at (p, j) is
  65536 + (p%4)*12500 + j on the vector engine, accumulate along free
  dim on the scalar engine.  A [128]->[32] fold matmul on the tensor
  engine sums each row's 4 quarters, giving val = 65536*cnt + idx
  exactly in fp32 (all quantities < 2^17, one-hot rows have <= one 1).

Phase B (gather + loss):
  cnt = (val >= 65536), idx = val - 65536*cnt.  Indices go to DRAM
  scratch in flat row order [vo(32) | vi(192) | neg(320)], are read
  back 128-rows-at-a-time, and drive single-offset indirect DMA
  gathers of U rows plus the per-row replicated vo V-row.  Per-row
  dots d = U_row . V_vo_row via DVE mult + ACT accumulate; then
  log-sigmoid terms via Exp/Log (one ACT table set) and per-batch
  reductions through small DRAM reshuffles.

Host: batch-shard across 8 cores, mean of the 256 per-batch terms.
"""
import numpy as np

import concourse.bass as bass
import concourse.mybir as mybir
from concourse.tile import TileContext
from concourse.bass_utils import run_bass_kernel_spmd

VOC = 50000
EMB = 300
B = 256
CTX = 6
K = 10
NCORES = 8
BPC = B // NCORES                    # 32 batch rows per core
NV = BPC * CTX                       # 192 vi rows per core
NN = BPC * K                         # 320 neg rows per core
NROWS = BPC + NV + NN                # 544 one-hot rows per core
NTILES = NROWS // 32                 # 17 extraction tiles of [128, 12500]
QW = VOC // 4                        # 12500 per partition-quarter
CH = QW // 2                         # 6250 free-dim chunk
NPAD = 640                           # padded flat row count (5 * 128)
NG = 5                               # gather tiles (4 full + 1 of 32 rows)
MARK = 65536.0                       # cnt marker (> max idx, power of 2)

F32 = mybir.dt.float32
I32 = mybir.dt.int32


def _split_multi_waits(nc):
    """This env's walrus accepts only ONE sync wait per instruction.
    Hoist extra waits into single-wait NoOps right before the owner."""
    cnt = 0
    for fn in nc.m.functions:
        for blk in fn.blocks:
            insts = list(blk.instructions)
            if not any(
                i.sync_info and i.sync_info.on_wait and len(i.sync_info.on_wait) > 1
                for i in insts
            ):
                continue
            new = []
            for inst in insts:
                si = inst.sync_info
                if si and si.on_wait and len(si.on_wait) > 1:
                    waits = list(si.on_wait)
                    for w in waits[:-1]:
                        cnt += 1
                        nop = mybir.InstNoOp(
                            name=f"mwsplit-{cnt}", engine=inst.engine, ins=[], outs=[]
                        )
                        nop.sync_info = mybir.SyncInfo(on_wait=[w], on_update=[])
                        new.append(nop)
                    inst.sync_info = mybir.SyncInfo(
                        on_wait=[waits[-1]], on_update=list(si.on_update or [])
                    )
                new.append(inst)
            blk.instructions = new
    return cnt


def _build():
    nc = bass.Bass(enable_partition_id=False)

    vo = nc.declare_dram_parameter("vo", [BPC, VOC], F32, isOutput=False)
    vi = nc.declare_dram_parameter("vi", [NV, VOC], F32, isOutput=False)
    ng = nc.declare_dram_parameter("ng", [NN, VOC], F32, isOutput=False)
    V = nc.declare_dram_parameter("V", [VOC, EMB], F32, isOutput=False)
    U = nc.declare_dram_parameter("U", [VOC, EMB], F32, isOutput=False)
    iota = nc.declare_dram_parameter("iota", [128, QW], F32, isOutput=False)
    foldq = nc.declare_dram_parameter("foldq", [128, 32], F32, isOutput=False)
    d_out = nc.declare_dram_parameter("dout", [128, NG], F32, isOutput=True)
    c_out = nc.declare_dram_parameter("cout", [32, NTILES], F32, isOutput=True)

    # per-tile [128, QW] sources: 4 partition-quarters per row
    srcs = [vo.rearrange("r (q f) -> (r q) f", q=4)]
    for u in range(CTX):
        srcs.append(vi[32 * u:32 * (u + 1), :].rearrange("r (q f) -> (r q) f", q=4))
    for u in range(K):
        srcs.append(ng[32 * u:32 * (u + 1), :].rearrange("r (q f) -> (r q) f", q=4))
    assert len(srcs) == NTILES

    with TileContext(nc) as tc:
        with (
            tc.tile_pool(name="const", bufs=1) as cpool,
            tc.tile_pool(name="data", bufs=3) as dpool,
            tc.tile_pool(name="prod", bufs=2) as ppool,
            tc.tile_pool(name="small", bufs=1) as spool,
            tc.tile_pool(name="gath", bufs=2) as gpool,
            tc.tile_pool(name="psum", bufs=1, space="PSUM") as psum_pool,
            tc.tile_pool(name="dram", bufs=1, space="DRAM") as dram_pool,
        ):
            iota_t = cpool.tile([128, QW], F32, tag="iota")
            nc.sync.dma_start(out=iota_t[:], in_=iota[:])
            foldq_t = cpool.tile([128, 32], F32, tag="foldq")
            nc.sync.dma_start(out=foldq_t[:], in_=foldq[:])

            # ---------------- Phase A: streaming extraction ----------------
            vals = spool.tile([128, NTILES * 2], F32, tag="vals")
            for t in range(NTILES):
                for h in range(2):
                    chunk = dpool.tile([128, CH], F32, tag="chunk")
                    nc.sync.dma_start(
                        out=chunk[:], in_=srcs[t][:, h * CH:(h + 1) * CH]
                    )
                    prod = ppool.tile([128, CH], F32, tag="prod")
                    nc.vector.tensor_tensor(
                        out=prod[:], in0=chunk[:],
                        in1=iota_t[:, h * CH:(h + 1) * CH],
                        op=mybir.AluOpType.mult,
                    )
                    col = 2 * t + h
                    nc.scalar.activation(
                        out=prod[:], in_=prod[:],
                        func=mybir.ActivationFunctionType.Copy,
                        accum_out=vals[:, col:col + 1],
                    )

            vals17 = spool.tile([128, NTILES], F32, tag="vals17")
            nc.vector.tensor_reduce(
                out=vals17[:], in_=vals[:].rearrange("p (t h) -> p t h", h=2),
                axis=mybir.AxisListType.X,
                op=mybir.AluOpType.add,
            )
            pvals = psum_pool.tile([32, NTILES], F32, tag="pvals")
            nc.tensor.matmul(
                out=pvals[:], lhsT=foldq_t[:], rhs=vals17[:], start=True, stop=True
            )
            # cnt = (val >= MARK), idx = val - MARK*cnt       [32, 17]
            cnt32 = spool.tile([32, NTILES], F32, tag="cnt32")
            nc.vector.tensor_scalar(
                out=cnt32[:], in0=pvals[:], scalar1=MARK, scalar2=None,
                op0=mybir.AluOpType.is_ge,
            )
            mk32 = spool.tile([32, NTILES], F32, tag="mk32")
            nc.vector.tensor_scalar(
                out=mk32[:], in0=cnt32[:], scalar1=MARK, scalar2=None,
                op0=mybir.AluOpType.mult,
            )
            idx32 = spool.tile([32, NTILES], F32, tag="idx32")
            nc.vector.tensor_tensor(
                out=idx32[:], in0=pvals[:], in1=mk32[:],
                op=mybir.AluOpType.subtract,
            )

            # ---------------- scratch round trips ----------------
            # flat row order: [vo(32) | vi(192) | neg(320)]; every DMA keeps
            # the DRAM-side innermost dim contiguous (DGE requirement here).
            sidx = dram_pool.tile([NROWS], F32, tag="sidx")
            svob = dram_pool.tile([NROWS], F32, tag="svob")
            for t in range(NTILES):
                nc.sync.dma_start(
                    out=sidx[32 * t:32 * (t + 1)].unsqueeze(1), in_=idx32[:, t:t + 1]
                )
            # voB offsets: vo idx replicated to every row's slot
            voidx = idx32[:, 0:1]
            rep6 = spool.tile([32, CTX], F32, tag="rep6")
            nc.vector.tensor_copy(out=rep6[:], in_=voidx.to_broadcast([32, CTX]))
            rep10 = spool.tile([32, K], F32, tag="rep10")
            nc.vector.tensor_copy(out=rep10[:], in_=voidx.to_broadcast([32, K]))
            nc.sync.dma_start(out=svob[:BPC].unsqueeze(1), in_=voidx)
            nc.sync.dma_start(
                out=svob[BPC:BPC + NV].rearrange("(b c) -> b c", c=CTX), in_=rep6[:]
            )
            nc.sync.dma_start(
                out=svob[BPC + NV:NROWS].rearrange("(b c) -> b c", c=K), in_=rep10[:]
            )

            # readbacks in gather-tile layout: rb[p, g] = flat[128 g + p]
            rb_idx = spool.tile([128, NG], F32, tag="rb_idx")
            rb_vob = spool.tile([128, NG], F32, tag="rb_vob")
            for g in range(NG):
                pg = 128 if g < NG - 1 else NROWS - 128 * (NG - 1)
                s = slice(128 * g, 128 * g + pg)
                nc.sync.dma_start(out=rb_idx[:pg, g:g + 1], in_=sidx[s].unsqueeze(1))
                nc.sync.dma_start(out=rb_vob[:pg, g:g + 1], in_=svob[s].unsqueeze(1))

            ofs_u = spool.tile([128, NG], I32, tag="ofs_u")
            nc.vector.tensor_copy(out=ofs_u[:], in_=rb_idx[:])
            ofs_v = spool.tile([128, NG], I32, tag="ofs_v")
            nc.vector.tensor_copy(out=ofs_v[:], in_=rb_vob[:])

            # ---------------- Phase B: gathers + per-row dots ----------------
            dall = spool.tile([128, NG], F32, tag="dall")
            nc.vector.memset(dall[:], 0.0)
            for g in range(NG):
                pg = 128 if g < NG - 1 else NROWS - 128 * (NG - 1)
                rowE = gpool.tile([128, EMB], F32, tag="rowE")
                nc.gpsimd.indirect_dma_start(
                    out=rowE[:pg, :], out_offset=None, in_=U[:],
                    in_offset=bass.IndirectOffsetOnAxis(ap=ofs_u[:pg, g:g + 1], axis=0),
                )
                voB = gpool.tile([128, EMB], F32, tag="voB")
                nc.gpsimd.indirect_dma_start(
                    out=voB[:pg, :], out_offset=None, in_=V[:],
                    in_offset=bass.IndirectOffsetOnAxis(ap=ofs_v[:pg, g:g + 1], axis=0),
                )
                prodB = gpool.tile([128, EMB], F32, tag="prodB")
                nc.vector.tensor_tensor(
                    out=prodB[:pg, :], in0=rowE[:pg, :], in1=voB[:pg, :],
                    op=mybir.AluOpType.mult,
                )
                nc.scalar.activation(
                    out=prodB[:pg, :], in_=prodB[:pg, :],
                    func=mybir.ActivationFunctionType.Copy,
                    accum_out=dall[:pg, g:g + 1],
                )

            nc.sync.dma_start(out=d_out[:], in_=dall[:])
            nc.sync.dma_start(out=c_out[:], in_=cnt32[:])

    _split_multi_waits(nc)
    return nc


def _consts():
    p = np.arange(128)
    iota_np = (
        MARK + (p % 4)[:, None] * QW + np.arange(QW)[None, :]
    ).astype(np.float32)
    foldq_np = np.zeros((128, 32), np.float32)
    foldq_np[p, p // 4] = 1.0
    return iota_np, foldq_np


_CACHE = {}


def kernel(vo, vi, neg_samples, V, U):
    if "nc" not in _CACHE:
        _CACHE["nc"] = _build()
        _CACHE["consts"] = _consts()
    nc = _CACHE["nc"]
    iota_np, foldq_np = _CACHE["consts"]

    vo = np.ascontiguousarray(vo, dtype=np.float32)
    vi = np.ascontiguousarray(vi, dtype=np.float32)
    neg = np.ascontiguousarray(neg_samples, dtype=np.float32)
    V = np.ascontiguousarray(V, dtype=np.float32)
    U = np.ascontiguousarray(U, dtype=np.float32)

    in_maps = []
    for c in range(NCORES):
        sl = slice(c * BPC, (c + 1) * BPC)
        in_maps.append({
            "vo": vo[sl],
            "vi": vi[sl].reshape(NV, VOC),
            "ng": neg[sl].reshape(NN, VOC),
            "V": V,
            "U": U,
            "iota": iota_np,
            "foldq": foldq_np,
        })

    res = run_bass_kernel_spmd(nc, in_maps, list(range(NCORES)))
    obs = []
    for r in res.results:
        d_flat = r["dout"].flatten(order="F")[:NROWS]
        c_flat = r["cout"].flatten(order="F")[:NROWS]
        d_vi = d_flat[BPC:BPC + NV].reshape(BPC, CTX)
        c_vi = c_flat[BPC:BPC + NV].reshape(BPC, CTX)
        d_ng = d_flat[BPC + NV:NROWS].reshape(BPC, K)
        lp = (d_vi * c_vi).sum(axis=1)
        ms = c_vi.sum(axis=1)
        x = lp / ms
        left = -np.log1p(np.exp(-x))
        right = (-np.log1p(np.exp(d_ng))).sum(axis=1)
        obs.append(-(left + right))
    ob = np.concatenate(obs)
    return np.float32(ob.mean(dtype=np.float64))



# revision 2
# speedup vs baseline: 1.1314x; 1.1314x over previous
"""CBOW negative-sampling loss kernel for 8 Trainium2 NeuronCores.

The reference computes one-hot @ table matmuls (embedding lookups in
disguise) followed by a tiny log-sigmoid loss.  Device-side algorithm
(v2 — single fused DVE pass, no DRAM scratch, no host iota table):

Streaming extraction:
  One-hot rows are streamed as 5 SBUF tiles of [128, *]:
    T0: the 32 vo rows split in 4 partition-quarters [128, 12500]
    T1..T4: the 192 vi + 320 neg rows as 4x [128, 50000]
  Every tile is read in [128, 6250] chunks.  One fused DVE
  tensor_tensor_reduce per chunk multiplies by an on-device iota tile
  (value MARK + j, MARK=65536) and row-reduces into vals[:, c].  Since
  each one-hot row has <= one 1, vals[p, c] = MARK + j exactly in fp32.
  Per tile, tiny DVE ops recover the hit chunk c = S2/max(S1,1) (S2 is
  a c-weighted column sum), the presence cnt = S1 >= MARK, and the
  in-row index off = 6250*c + cnt*(S1 - MARK).  For T0 a [128]->[32]
  fold matmul sums the 4 quarter contributions cnt*(MARK + 12500*q) +
  off into the global vo index.

Gather + dots (overlapped with streaming of later tiles):
  V rows for vo are gathered once [32, 300] and replicated to each
  tile's partition order by a one-hot [32->128] matmul into PSUM.  U
  rows are gathered per tile with single-offset indirect DMA driven
  straight from the extracted SBUF indices (no DRAM round trip).  A
  fused DVE multiply-reduce forms d = U_row . V_vo_row per partition.

Host: batch-shard across 8 cores, log-sigmoid loss terms + mean of the
256 per-batch terms (same split as the v1 baseline).
"""
import numpy as np

import concourse.bass as bass
import concourse.mybir as mybir
from concourse.tile import TileContext
from concourse.bass_utils import run_bass_kernel_spmd

VOC = 50000
EMB = 300
B = 256
CTX = 6
K = 10
NCORES = 8
BPC = B // NCORES                    # 32 batch rows per core
NV = BPC * CTX                       # 192 vi rows per core
NN = BPC * K                         # 320 neg rows per core
CH = 6250                            # free-dim chunk width
NCH = VOC // CH                      # 8 chunks per full 50000 row
QW = VOC // 4                        # 12500 per vo partition-quarter
MARK = 65536.0                       # presence marker (> max idx, power of 2)

F32 = mybir.dt.float32
I32 = mybir.dt.int32


def _split_multi_waits(nc):
    """This env's walrus accepts only ONE sync wait per instruction.
    Hoist extra waits into single-wait NoOps right before the owner."""
    cnt = 0
    for fn in nc.m.functions:
        for blk in fn.blocks:
            insts = list(blk.instructions)
            if not any(
                i.sync_info and i.sync_info.on_wait and len(i.sync_info.on_wait) > 1
                for i in insts
            ):
                continue
            new = []
            for inst in insts:
                si = inst.sync_info
                if si and si.on_wait and len(si.on_wait) > 1:
                    waits = list(si.on_wait)
                    for w in waits[:-1]:
                        cnt += 1
                        nop = mybir.InstNoOp(
                            name=f"mwsplit-{cnt}", engine=inst.engine, ins=[], outs=[]
                        )
                        nop.sync_info = mybir.SyncInfo(on_wait=[w], on_update=[])
                        new.append(nop)
                    inst.sync_info = mybir.SyncInfo(
                        on_wait=[waits[-1]], on_update=list(si.on_update or [])
                    )
                new.append(inst)
            blk.instructions = new
    return cnt


def _build():
    nc = bass.Bass(enable_partition_id=False)

    vo = nc.declare_dram_parameter("vo", [BPC, VOC], F32, isOutput=False)
    vi = nc.declare_dram_parameter("vi", [NV, VOC], F32, isOutput=False)
    ng = nc.declare_dram_parameter("ng", [NN, VOC], F32, isOutput=False)
    V = nc.declare_dram_parameter("V", [VOC, EMB], F32, isOutput=False)
    U = nc.declare_dram_parameter("U", [VOC, EMB], F32, isOutput=False)
    # merged consts: col 0 qbaseM | 1:3 wc2-bit0 | 3:27 wc8 bits | 27:111
    # wc28 bits | 111:143 foldq
    cc = nc.declare_dram_parameter("cc", [128, 143], F32, isOutput=False)
    reps = nc.declare_dram_parameter("reps", [32, 4 * 128], F32, isOutput=False)
    d_out = nc.declare_dram_parameter("dout", [128, 4], F32, isOutput=True)
    c_out = nc.declare_dram_parameter("cout", [128, 4], F32, isOutput=True)

    vo_q = vo.rearrange("r (q f) -> (r q) f", q=4)     # [128, 12500]
    # big tiles: list of (dram slice, partition range) DMAs per tile
    big = [
        [(vi[0:128, :], 0, 128)],
        [(vi[128:NV, :], 0, 64), (ng[0:64, :], 64, 128)],
        [(ng[64:192, :], 0, 128)],
        [(ng[192:NN, :], 0, 128)],
    ]

    AX = mybir.AxisListType.X
    OP = mybir.AluOpType
    ACTF = mybir.ActivationFunctionType

    with TileContext(nc) as tc:
        with (
            tc.tile_pool(name="const", bufs=1) as cpool,
            tc.tile_pool(name="data", bufs=3) as dpool,
            tc.tile_pool(name="prod", bufs=1) as ppool,
            tc.tile_pool(name="pieces", bufs=12) as qpool,
            tc.tile_pool(name="small", bufs=2) as spool,
            tc.tile_pool(name="keep", bufs=1) as kpool,
            tc.tile_pool(name="gath", bufs=2) as gpool,
            tc.tile_pool(name="psum", bufs=2, space="PSUM") as psum_pool,
        ):
            # on-device iota (value MARK + j, exact in fp32); consts go on
            # the ACT DMA queue so SP can start the big streaming DMAs at
            # once (keeps the serial DMA device fed from t=0)
            iota_t = cpool.tile([128, CH], F32, tag="iota")
            nc.gpsimd.iota(
                out=iota_t[:], pattern=[[1, CH]], base=int(MARK),
                channel_multiplier=0, allow_small_or_imprecise_dtypes=True,
            )
            dall = kpool.tile([128, 4], F32, tag="dall")
            call = kpool.tile([128, 4], F32, tag="call")

            def stream_tile(srcs, nch, vals, split_from=None):
                """DMA [128, CH] chunks + fused multiply-reduce each into
                vals[:, col].  Chunks >= split_from are 5 piecewise
                [128, 1250] DMAs+reduces (one vals column each, weight
                handled by the wc table) so DVE tracks the DMA closely and
                the kernel-tail dependency chain stays short."""
                if split_from is None:
                    split_from = nch
                col = 0
                for c in range(split_from):
                    chunk = dpool.tile([128, CH], F32, tag="chunk")
                    for src, p0, p1 in srcs:
                        nc.sync.dma_start(
                            out=chunk[p0:p1, :], in_=src[:, c * CH:(c + 1) * CH]
                        )
                    prod = ppool.tile([128, CH], F32, tag="prod")
                    nc.vector.scalar_tensor_tensor(
                        out=prod[:], in0=chunk[:], scalar=1.0, in1=iota_t[:],
                        op0=OP.mult, op1=OP.mult,
                        accum_out=vals[:, col:col + 1],
                    )
                    col += 1
                W = CH // 5
                for c in range(split_from, nch):
                    for h in range(5):
                        j0 = h * W
                        piece = qpool.tile([128, W], F32, tag="piece")
                        for src, p0, p1 in srcs:
                            nc.sync.dma_start(
                                out=piece[p0:p1, :],
                                in_=src[:, c * CH + j0:c * CH + j0 + W],
                            )
                        prodp = ppool.tile([128, W], F32, tag="prodp")
                        nc.vector.scalar_tensor_tensor(
                            out=prodp[:], in0=piece[:], scalar=1.0,
                            in1=iota_t[:, j0:j0 + W],
                            op0=OP.mult, op1=OP.mult,
                            accum_out=vals[:, col:col + 1],
                        )
                        col += 1

            def extract(vals, wcbits, ncol, cnt_out, off_out, tag):
                """cnt = (row had a 1); off = CH*c_hit + j_hit (0 if none).
                The hit-chunk index c is recovered bit-by-bit: one weighted
                column reduce per bit of c (weight = that bit of each
                column's chunk index), then is_ge(MARK) -> bit, scaled by
                CH*2^b and summed.  No division needed anywhere.  Heavy
                reduces on DVE (no DMA deps); scalar chain on Pool."""
                S1 = spool.tile([128, 1], F32, tag=f"S1{tag}")
                nc.vector.tensor_reduce(out=S1[:], in_=vals[:], axis=AX, op=OP.add)
                bms = []
                for b, wc in enumerate(wcbits):
                    junk = spool.tile([128, ncol], F32, tag=f"jk{ncol}")
                    S2 = spool.tile([128, 1], F32, tag=f"S2{tag}{b}")
                    nc.vector.scalar_tensor_tensor(
                        out=junk[:], in0=vals[:], scalar=1.0, in1=wc,
                        op0=OP.mult, op1=OP.mult, accum_out=S2[:],
                    )
                    bb = spool.tile([128, 1], F32, tag=f"bb{tag}{b}")
                    nc.gpsimd.tensor_scalar(
                        out=bb[:], in0=S2[:], scalar1=MARK, scalar2=None,
                        op0=OP.is_ge,
                    )
                    bm = spool.tile([128, 1], F32, tag=f"bm{tag}{b}")
                    nc.gpsimd.tensor_scalar(
                        out=bm[:], in0=bb[:], scalar1=float(CH * (1 << b)),
                        scalar2=None, op0=OP.mult,
                    )
                    bms.append(bm)
                nc.gpsimd.tensor_scalar(
                    out=cnt_out, in0=S1[:], scalar1=MARK, scalar2=None, op0=OP.is_ge
                )
                acc = bms[0]
                for b in range(1, len(bms)):
                    nacc = spool.tile([128, 1], F32, tag=f"ac{tag}{b}")
                    nc.gpsimd.tensor_tensor(out=nacc[:], in0=acc[:], in1=bms[b][:],
                                            op=OP.add)
                    acc = nacc
                jp = spool.tile([128, 1], F32, tag=f"jp{tag}")
                nc.gpsimd.tensor_scalar(
                    out=jp[:], in0=S1[:], scalar1=MARK, scalar2=None, op0=OP.subtract
                )
                cj = spool.tile([128, 1], F32, tag=f"cj{tag}")
                nc.gpsimd.tensor_tensor(out=cj[:], in0=cnt_out, in1=jp[:], op=OP.mult)
                nc.gpsimd.tensor_tensor(out=off_out, in0=acc[:], in1=cj[:], op=OP.add)

            # ---------------- T0: vo quarters ----------------
            vals0 = spool.tile([128, 2], F32, tag="vals0")
            stream_tile([(vo_q, 0, 128)], 2, vals0)

            # small constants: two merged DMAs on the ACT queue (after the
            # first big chunks, one HWDGE generation each)
            cc_t = cpool.tile([128, 143], F32, tag="cc")
            nc.scalar.dma_start(out=cc_t[:], in_=cc[:])
            reps_all = cpool.tile([32, 4 * 128], F32, tag="reps")
            nc.scalar.dma_start(out=reps_all[:], in_=reps[:])
            qbaseM_t = cc_t[:, 0:1]
            wc2_bits = [cc_t[:, 1:3]]
            wc8_bits = [cc_t[:, 3 + 8 * b:11 + 8 * b] for b in range(3)]
            wc28_bits = [cc_t[:, 27 + 28 * b:55 + 28 * b] for b in range(3)]
            foldq_t = cc_t[:, 111:143]
            reps_t = [reps_all[:, 128 * t:128 * (t + 1)] for t in range(4)]

            cnt0 = spool.tile([128, 1], F32, tag="cnt0")
            off0 = spool.tile([128, 1], F32, tag="off0")
            extract(vals0, wc2_bits, 2, cnt0[:], off0[:], "t0")
            # X = cnt*(MARK + 12500 q) + off ; fold quarters -> [32, 1]
            xq = spool.tile([128, 1], F32, tag="xq")
            nc.gpsimd.tensor_tensor(out=xq[:], in0=cnt0[:], in1=qbaseM_t, op=OP.mult)
            x2 = spool.tile([128, 1], F32, tag="x2")
            nc.gpsimd.tensor_tensor(out=x2[:], in0=xq[:], in1=off0[:], op=OP.add)
            pfold = psum_pool.tile([32, 1], F32, tag="p32")
            nc.tensor.matmul(out=pfold[:], lhsT=foldq_t, rhs=x2[:],
                             start=True, stop=True)
            idxvo = spool.tile([32, 1], F32, tag="idxvo")
            nc.vector.tensor_scalar(
                out=idxvo[:], in0=pfold[:], scalar1=MARK, scalar2=None,
                op0=OP.subtract,
            )
            ofs_v = spool.tile([32, 1], I32, tag="ofsv")
            nc.gpsimd.tensor_copy(out=ofs_v[:], in_=idxvo[:])
            voV = cpool.tile([32, EMB], F32, tag="voV")
            nc.gpsimd.indirect_dma_start(
                out=voV[:], out_offset=None, in_=V[:],
                in_offset=bass.IndirectOffsetOnAxis(ap=ofs_v[:], axis=0),
            )

            # ---------------- T1..T4: vi + neg rows ----------------
            # The per-tile dot runs as Pool multiply + ACT accumulate —
            # neither touches the DVE streaming queue, so a gather that
            # lands late can only block Pool (whose next deadline is a
            # full tile away).
            for t in range(4):
                last = t == 3
                ncol = 3 + 25 if last else NCH
                vals = spool.tile([128, ncol], F32, tag=f"vals{ncol}")
                stream_tile(big[t], NCH, vals, split_from=3 if last else None)
                off = spool.tile([128, 1], F32, tag="off")
                extract(vals, wc28_bits if last else wc8_bits, ncol,
                        call[:, t:t + 1], off[:], "tb")
                ofs_u = spool.tile([128, 1], I32, tag="ofsu")
                nc.gpsimd.tensor_copy(out=ofs_u[:], in_=off[:])
                rowU = gpool.tile([128, EMB], F32, tag="rowU")
                nc.gpsimd.indirect_dma_start(
                    out=rowU[:], out_offset=None, in_=U[:],
                    in_offset=bass.IndirectOffsetOnAxis(ap=ofs_u[:], axis=0),
                )
                pB = psum_pool.tile([128, EMB], F32, tag="pB")
                nc.tensor.matmul(out=pB[:], lhsT=reps_t[t], rhs=voV[:],
                                 start=True, stop=True)
                voB = gpool.tile([128, EMB], F32, tag="voB")
                nc.scalar.activation(out=voB[:], in_=pB[:], func=ACTF.Copy)
                prodB = gpool.tile([128, EMB], F32, tag="prodB")
                nc.gpsimd.tensor_tensor(
                    out=prodB[:], in0=rowU[:], in1=voB[:], op=OP.mult
                )
                nc.scalar.activation(
                    out=prodB[:], in_=prodB[:], func=ACTF.Copy,
                    accum_out=dall[:, t:t + 1],
                )

            # both outputs ride the ACT queue: d_out directly follows the
            # last accumulate there (no cross-engine hop on the tail)
            nc.scalar.dma_start(out=c_out[:], in_=call[:])
            nc.scalar.dma_start(out=d_out[:], in_=dall[:])

    _split_multi_waits(nc)
    mybir.codegen_inst_isa_subclasses(nc)
    return nc


def _consts():
    p = np.arange(128)
    qbaseM = (MARK + (p % 4) * QW).astype(np.float32).reshape(128, 1)
    wc2 = np.tile(np.arange(2, dtype=np.float32), (128, 1))
    wc8 = np.tile(np.arange(8, dtype=np.float32), (128, 1))
    c8 = np.arange(8)
    c28 = np.concatenate([np.arange(3), np.repeat(np.arange(3, 8), 5)])
    wc8b = np.concatenate(
        [np.tile(((c8 >> b) & 1).astype(np.float32), (128, 1)) for b in range(3)],
        axis=1)
    wc28b = np.concatenate(
        [np.tile(((c28 >> b) & 1).astype(np.float32), (128, 1)) for b in range(3)],
        axis=1)
    foldq = np.zeros((128, 32), np.float32)
    foldq[p, p // 4] = 1.0
    cc = np.concatenate([qbaseM, wc2, wc8b, wc28b, foldq], axis=1)
    # reps[t, b, p] = 1 iff partition p of tile t holds a row of batch b
    bmap = np.empty((4, 128), np.int64)
    bmap[0] = p // CTX                                   # vi rows 0..127
    bmap[1, :64] = (128 + p[:64]) // CTX                 # vi rows 128..191
    bmap[1, 64:] = (p[64:] - 64) // K                    # ng rows 0..63
    bmap[2] = (64 + p) // K                              # ng rows 64..191
    bmap[3] = (192 + p) // K                             # ng rows 192..319
    reps = np.zeros((4, 32, 128), np.float32)
    for t in range(4):
        reps[t, bmap[t], p] = 1.0
    reps = reps.transpose(1, 0, 2).reshape(32, 4 * 128)
    return cc, reps


_CACHE = {}


def kernel(vo, vi, neg_samples, V, U):
    if "nc" not in _CACHE:
        _CACHE["nc"] = _build()
        _CACHE["consts"] = _consts()
    nc = _CACHE["nc"]
    cc, reps = _CACHE["consts"]

    vo = np.ascontiguousarray(vo, dtype=np.float32)
    vi = np.ascontiguousarray(vi, dtype=np.float32)
    neg = np.ascontiguousarray(neg_samples, dtype=np.float32)
    V = np.ascontiguousarray(V, dtype=np.float32)
    U = np.ascontiguousarray(U, dtype=np.float32)

    in_maps = []
    for c in range(NCORES):
        sl = slice(c * BPC, (c + 1) * BPC)
        in_maps.append({
            "vo": vo[sl],
            "vi": vi[sl].reshape(NV, VOC),
            "ng": neg[sl].reshape(NN, VOC),
            "V": V,
            "U": U,
            "cc": cc, "reps": reps,
        })

    res = run_bass_kernel_spmd(nc, in_maps, list(range(NCORES)))
    obs = []
    for r in res.results:
        d = r["dout"]                                  # [128, 4]
        cc = r["cout"]                                 # [128, 4]
        d_vi = np.concatenate([d[:, 0], d[:64, 1]]).reshape(BPC, CTX)
        c_vi = np.concatenate([cc[:, 0], cc[:64, 1]]).reshape(BPC, CTX)
        d_ng = np.concatenate([d[64:, 1], d[:, 2], d[:, 3]]).reshape(BPC, K)
        lp = (d_vi * c_vi).sum(axis=1)
        ms = c_vi.sum(axis=1)
        x = lp / ms
        left = -np.log1p(np.exp(-x))
        right = (-np.log1p(np.exp(d_ng))).sum(axis=1)
        obs.append(-(left + right))
    ob = np.concatenate(obs)
    return np.float32(ob.mean(dtype=np.float64))


# revision 3
# speedup vs baseline: 1.1386x; 1.0064x over previous
"""CBOW negative-sampling loss kernel for 8 Trainium2 NeuronCores.

The reference computes one-hot @ table matmuls (embedding lookups in
disguise) followed by a tiny log-sigmoid loss.  Device-side algorithm
(v2: single fused DVE pass per byte, no DRAM scratch, no host iota):

Streaming extraction (DMA-bound, ~305us/core at the 360 GB/s limit):
  One-hot rows stream as 5 SBUF tiles of [128, *]:
    T0: the 32 vo rows split in 4 partition-quarters [128, 12500]
    T1..T4: the 192 vi + 320 neg rows as 4x [128, 50000]
  Chunks are [128, 6250]; T4's last 5 chunks are split into [128, 1250]
  pieces (last one 2x625) so the DVE tracks the DMA to the final byte.
  ONE fused DVE scalar_tensor_tensor per chunk multiplies by a
  Pool-generated iota tile (value MARK + j, MARK = 65536) and
  row-reduces into vals[:, col]; each one-hot row has <= one 1, so
  vals[p, col] = MARK + in-chunk-offset, exactly in fp32.
  Per tile the hit chunk c is recovered BIT-WISE (no HW divide): for
  each bit b, a weighted column reduce with weights bit_b(chunk(col))
  gives S2_b; bit_b = S2_b >= MARK.  Then
    off = max(sum_b 6250*2^b*bit_b + (S1 - MARK), 0),   cnt = S1 >= MARK
  all on DVE (no DMA deps, so the in-order DVE queue never stalls).
  For T0 a [128]->[32] one-hot fold matmul sums the 4 quarter
  contributions cnt*(MARK + 12500 q) + off into the global vo index.

Gathers + dots (overlapped with streaming of later tiles):
  V rows for vo are gathered once [32, 300] and replicated to each
  tile's partition order by a one-hot [32->128] matmul into PSUM (ACT
  copies PSUM->SBUF).  U rows are gathered per tile with single-offset
  indirect DMA driven straight from the extracted SBUF indices.  The
  per-row dot d = U_row . V_vo_row runs as Pool multiply + ACT
  accumulate mid-stream (never on the DVE queue, which would stall
  behind the gather), and as one fused DVE op for the final tile.

Host: batch-shard across 8 cores, log-sigmoid loss terms + mean of the
256 per-batch terms (same split as the v1 baseline).

Engine/ISA notes (hardware-verified): tensor_tensor_reduce (bass_isa
extended ISA) compiles but faults at runtime here - scalar_tensor_tensor
(core BIR, is_scalar_tensor_tensor=True) is the fused multiply+reduce
that actually runs.  Pool supports tensor_tensor {mult,add,sub} and
tensor_scalar {mult,sub,max,is_ge} only (no divide anywhere).
tensor_scalar with accum_out faults at runtime.
"""
import numpy as np

import concourse.bass as bass
import concourse.mybir as mybir
from concourse.tile import TileContext
from concourse.bass_utils import run_bass_kernel_spmd

VOC = 50000
EMB = 300
B = 256
CTX = 6
K = 10
NCORES = 8
BPC = B // NCORES                    # 32 batch rows per core
NV = BPC * CTX                       # 192 vi rows per core
NN = BPC * K                         # 320 neg rows per core
CH = 6250                            # free-dim chunk width
NCH = VOC // CH                      # 8 chunks per full 50000 row
QW = VOC // 4                        # 12500 per vo partition-quarter
MARK = 65536.0                       # presence marker (> max idx, power of 2)

F32 = mybir.dt.float32
I32 = mybir.dt.int32


def _split_multi_waits(nc):
    """This env's walrus accepts only ONE sync wait per instruction.
    Hoist extra waits into single-wait NoOps right before the owner."""
    cnt = 0
    for fn in nc.m.functions:
        for blk in fn.blocks:
            insts = list(blk.instructions)
            if not any(
                i.sync_info and i.sync_info.on_wait and len(i.sync_info.on_wait) > 1
                for i in insts
            ):
                continue
            new = []
            for inst in insts:
                si = inst.sync_info
                if si and si.on_wait and len(si.on_wait) > 1:
                    waits = list(si.on_wait)
                    for w in waits[:-1]:
                        cnt += 1
                        nop = mybir.InstNoOp(
                            name=f"mwsplit-{cnt}", engine=inst.engine, ins=[], outs=[]
                        )
                        nop.sync_info = mybir.SyncInfo(on_wait=[w], on_update=[])
                        new.append(nop)
                    inst.sync_info = mybir.SyncInfo(
                        on_wait=[waits[-1]], on_update=list(si.on_update or [])
                    )
                new.append(inst)
            blk.instructions = new
    return cnt


def _build():
    nc = bass.Bass(enable_partition_id=False)

    vo = nc.declare_dram_parameter("vo", [BPC, VOC], F32, isOutput=False)
    vi = nc.declare_dram_parameter("vi", [NV, VOC], F32, isOutput=False)
    ng = nc.declare_dram_parameter("ng", [NN, VOC], F32, isOutput=False)
    V = nc.declare_dram_parameter("V", [VOC, EMB], F32, isOutput=False)
    U = nc.declare_dram_parameter("U", [VOC, EMB], F32, isOutput=False)
    # merged consts: col 0 qbaseM | 1:3 wc2-bit0 | 3:27 wc8 bits | 27:114
    # wc29 bits | 114:146 foldq
    cc = nc.declare_dram_parameter("cc", [128, 146], F32, isOutput=False)
    reps = nc.declare_dram_parameter("reps", [32, 4 * 128], F32, isOutput=False)
    d_out = nc.declare_dram_parameter("dout", [128, 4], F32, isOutput=True)
    c_out = nc.declare_dram_parameter("cout", [128, 4], F32, isOutput=True)

    vo_q = vo.rearrange("r (q f) -> (r q) f", q=4)     # [128, 12500]
    # big tiles: list of (dram slice, partition range) DMAs per tile
    big = [
        [(vi[0:128, :], 0, 128)],
        [(vi[128:NV, :], 0, 64), (ng[0:64, :], 64, 128)],
        [(ng[64:192, :], 0, 128)],
        [(ng[192:NN, :], 0, 128)],
    ]

    AX = mybir.AxisListType.X
    OP = mybir.AluOpType
    ACTF = mybir.ActivationFunctionType

    with TileContext(nc) as tc:
        with (
            tc.tile_pool(name="const", bufs=1) as cpool,
            tc.tile_pool(name="data", bufs=3) as dpool,
            tc.tile_pool(name="prod", bufs=1) as ppool,
            tc.tile_pool(name="pieces", bufs=12) as qpool,
            tc.tile_pool(name="pieces2", bufs=2) as q2pool,
            tc.tile_pool(name="small", bufs=2) as spool,
            tc.tile_pool(name="keep", bufs=1) as kpool,
            tc.tile_pool(name="gath", bufs=2) as gpool,
            tc.tile_pool(name="psum", bufs=2, space="PSUM") as psum_pool,
        ):
            # on-device iota (value MARK + j, exact in fp32); consts go on
            # the ACT DMA queue so SP can start the big streaming DMAs at
            # once (keeps the serial DMA device fed from t=0)
            iota_t = cpool.tile([128, CH], F32, tag="iota")
            nc.gpsimd.iota(
                out=iota_t[:], pattern=[[1, CH]], base=int(MARK),
                channel_multiplier=0, allow_small_or_imprecise_dtypes=True,
            )
            dall = kpool.tile([128, 4], F32, tag="dall")
            call = kpool.tile([128, 4], F32, tag="call")

            def stream_tile(srcs, nch, vals, split_from=None):
                """DMA [128, CH] chunks + fused multiply-reduce each into
                vals[:, col].  Chunks >= split_from are 5 piecewise
                [128, 1250] DMAs+reduces (one vals column each, weight
                handled by the wc table) so DVE tracks the DMA closely and
                the kernel-tail dependency chain stays short."""
                if split_from is None:
                    split_from = nch
                col = 0
                for c in range(split_from):
                    chunk = dpool.tile([128, CH], F32, tag="chunk")
                    for src, p0, p1 in srcs:
                        nc.sync.dma_start(
                            out=chunk[p0:p1, :], in_=src[:, c * CH:(c + 1) * CH]
                        )
                    prod = ppool.tile([128, CH], F32, tag="prod")
                    nc.vector.scalar_tensor_tensor(
                        out=prod[:], in0=chunk[:], scalar=1.0, in1=iota_t[:],
                        op0=OP.mult, op1=OP.mult,
                        accum_out=vals[:, col:col + 1],
                    )
                    col += 1
                W = CH // 5
                subs = []
                for c in range(split_from, nch):
                    for h in range(5):
                        if c == nch - 1 and h == 4:
                            subs.extend([(c, h * W, W // 2), (c, h * W + W // 2, W // 2)])
                        else:
                            subs.append((c, h * W, W))
                for c, j0, w in subs:
                    pp = qpool if w == W else q2pool
                    piece = pp.tile([128, w], F32, tag=f"piece{w}")
                    for src, p0, p1 in srcs:
                        nc.sync.dma_start(
                            out=piece[p0:p1, :],
                            in_=src[:, c * CH + j0:c * CH + j0 + w],
                        )
                    prodp = ppool.tile([128, w], F32, tag=f"prodp{w}")
                    nc.vector.scalar_tensor_tensor(
                        out=prodp[:], in0=piece[:], scalar=1.0,
                        in1=iota_t[:, j0:j0 + w],
                        op0=OP.mult, op1=OP.mult,
                        accum_out=vals[:, col:col + 1],
                    )
                    col += 1

            def extract(vals, wcbits, ncol, cnt_out, off_out, tag,
                        pre_cols=None):
                """cnt = (row had a 1); off = CH*c_hit + j_hit (0 if none).
                The hit-chunk index c is recovered bit-by-bit: one weighted
                column reduce per bit of c (weight = that bit of each
                column's chunk index), then is_ge(MARK) -> bit, scaled by
                CH*2^b and summed.  No division needed anywhere.  Heavy
                reduces on DVE (no DMA deps); scalar chain on Pool."""
                if pre_cols is not None:
                    # partial reduces over the early columns overlap the
                    # final streamed pieces; the tail then only adds the
                    # last column (whose chunk index has all bits set)
                    S1p = spool.tile([128, 1], F32, tag=f"S1p{tag}")
                    nc.vector.tensor_reduce(
                        out=S1p[:], in_=vals[:, :pre_cols], axis=AX, op=OP.add)
                    S1 = spool.tile([128, 1], F32, tag=f"S1{tag}")
                    nc.vector.tensor_tensor(
                        out=S1[:], in0=S1p[:], in1=vals[:, ncol - 1:ncol],
                        op=OP.add)
                else:
                    S1 = spool.tile([128, 1], F32, tag=f"S1{tag}")
                    nc.vector.tensor_reduce(out=S1[:], in_=vals[:], axis=AX,
                                            op=OP.add)
                jp = spool.tile([128, 1], F32, tag=f"jp{tag}")
                nc.vector.tensor_scalar(
                    out=jp[:], in0=S1[:], scalar1=MARK, scalar2=None, op0=OP.subtract
                )
                bms = []
                for b, wc in enumerate(wcbits):
                    S2 = spool.tile([128, 1], F32, tag=f"S2{tag}{b}")
                    if pre_cols is not None:
                        junk = spool.tile([128, pre_cols], F32, tag=f"jkp{tag}")
                        S2p = spool.tile([128, 1], F32, tag=f"S2p{tag}{b}")
                        nc.vector.scalar_tensor_tensor(
                            out=junk[:], in0=vals[:, :pre_cols], scalar=1.0,
                            in1=wc[:, :pre_cols],
                            op0=OP.mult, op1=OP.mult, accum_out=S2p[:],
                        )
                        nc.vector.tensor_tensor(
                            out=S2[:], in0=S2p[:], in1=vals[:, ncol - 1:ncol],
                            op=OP.add)
                    else:
                        junk = spool.tile([128, ncol], F32, tag=f"jk{ncol}")
                        nc.vector.scalar_tensor_tensor(
                            out=junk[:], in0=vals[:], scalar=1.0, in1=wc,
                            op0=OP.mult, op1=OP.mult, accum_out=S2[:],
                        )
                    bb = spool.tile([128, 1], F32, tag=f"bb{tag}{b}")
                    nc.vector.tensor_scalar(
                        out=bb[:], in0=S2[:], scalar1=MARK, scalar2=None,
                        op0=OP.is_ge,
                    )
                    bm = spool.tile([128, 1], F32, tag=f"bm{tag}{b}")
                    nc.vector.tensor_scalar(
                        out=bm[:], in0=bb[:], scalar1=float(CH * (1 << b)),
                        scalar2=None, op0=OP.mult,
                    )
                    bms.append(bm)
                nc.vector.tensor_scalar(
                    out=cnt_out, in0=S1[:], scalar1=MARK, scalar2=None, op0=OP.is_ge
                )
                acc = bms[0]
                for b in range(1, len(bms)):
                    nacc = spool.tile([128, 1], F32, tag=f"ac{tag}{b}")
                    nc.vector.tensor_tensor(out=nacc[:], in0=acc[:], in1=bms[b][:],
                                             op=OP.add)
                    acc = nacc
                # off = max(acc + (S1 - MARK), 0): a no-hit row gives
                # acc=0, S1=0 -> clamps to 0; keeps cnt off this path
                aj = spool.tile([128, 1], F32, tag=f"aj{tag}")
                nc.vector.tensor_tensor(out=aj[:], in0=acc[:], in1=jp[:], op=OP.add)
                nc.vector.tensor_scalar(
                    out=off_out, in0=aj[:], scalar1=0.0, scalar2=None, op0=OP.max
                )

            # ---------------- T0: vo quarters ----------------
            vals0 = spool.tile([128, 2], F32, tag="vals0")
            stream_tile([(vo_q, 0, 128)], 2, vals0)

            # small constants: two merged DMAs on the ACT queue (after the
            # first big chunks, one HWDGE generation each)
            cc_t = cpool.tile([128, 146], F32, tag="cc")
            nc.scalar.dma_start(out=cc_t[:], in_=cc[:])
            reps_all = cpool.tile([32, 4 * 128], F32, tag="reps")
            nc.scalar.dma_start(out=reps_all[:], in_=reps[:])
            qbaseM_t = cc_t[:, 0:1]
            wc2_bits = [cc_t[:, 1:3]]
            wc8_bits = [cc_t[:, 3 + 8 * b:11 + 8 * b] for b in range(3)]
            wc28_bits = [cc_t[:, 27 + 29 * b:56 + 29 * b] for b in range(3)]
            foldq_t = cc_t[:, 114:146]
            reps_t = [reps_all[:, 128 * t:128 * (t + 1)] for t in range(4)]

            cnt0 = spool.tile([128, 1], F32, tag="cnt0")
            off0 = spool.tile([128, 1], F32, tag="off0")
            extract(vals0, wc2_bits, 2, cnt0[:], off0[:], "t0")
            # X = cnt*(MARK + 12500 q) + off ; fold quarters -> [32, 1]
            xq = spool.tile([128, 1], F32, tag="xq")
            nc.vector.tensor_tensor(out=xq[:], in0=cnt0[:], in1=qbaseM_t, op=OP.mult)
            x2 = spool.tile([128, 1], F32, tag="x2")
            nc.vector.tensor_tensor(out=x2[:], in0=xq[:], in1=off0[:], op=OP.add)
            pfold = psum_pool.tile([32, 1], F32, tag="p32")
            nc.tensor.matmul(out=pfold[:], lhsT=foldq_t, rhs=x2[:],
                             start=True, stop=True)
            idxvo = spool.tile([32, 1], F32, tag="idxvo")
            nc.vector.tensor_scalar(
                out=idxvo[:], in0=pfold[:], scalar1=MARK, scalar2=None,
                op0=OP.subtract,
            )
            ofs_v = spool.tile([32, 1], I32, tag="ofsv")
            nc.vector.tensor_copy(out=ofs_v[:], in_=idxvo[:])
            voV = cpool.tile([32, EMB], F32, tag="voV")
            nc.gpsimd.indirect_dma_start(
                out=voV[:], out_offset=None, in_=V[:],
                in_offset=bass.IndirectOffsetOnAxis(ap=ofs_v[:], axis=0),
            )

            # ---------------- T1..T4: vi + neg rows ----------------
            # The per-tile dot runs as Pool multiply + ACT accumulate —
            # neither touches the DVE streaming queue, so a gather that
            # lands late can only block Pool (whose next deadline is a
            # full tile away).
            for t in range(4):
                last = t == 3
                ncol = 3 + 26 if last else NCH
                vals = spool.tile([128, ncol], F32, tag=f"vals{ncol}")
                stream_tile(big[t], NCH, vals, split_from=3 if last else None)
                off = spool.tile([128, 1], F32, tag="off")
                extract(vals, wc28_bits if last else wc8_bits, ncol,
                        call[:, t:t + 1], off[:], "tb")
                ofs_u = spool.tile([128, 1], I32, tag="ofsu")
                nc.vector.tensor_copy(out=ofs_u[:], in_=off[:])
                rowU = gpool.tile([128, EMB], F32, tag="rowU")
                nc.gpsimd.indirect_dma_start(
                    out=rowU[:], out_offset=None, in_=U[:],
                    in_offset=bass.IndirectOffsetOnAxis(ap=ofs_u[:], axis=0),
                )
                pB = psum_pool.tile([128, EMB], F32, tag="pB")
                nc.tensor.matmul(out=pB[:], lhsT=reps_t[t], rhs=voV[:],
                                 start=True, stop=True)
                voB = gpool.tile([128, EMB], F32, tag="voB")
                nc.scalar.activation(out=voB[:], in_=pB[:], func=ACTF.Copy)
                prodB = gpool.tile([128, EMB], F32, tag="prodB")
                if last:
                    # tail: one fused DVE op (DVE is idle by now)
                    nc.vector.scalar_tensor_tensor(
                        out=prodB[:], in0=rowU[:], scalar=1.0, in1=voB[:],
                        op0=OP.mult, op1=OP.mult, accum_out=dall[:, t:t + 1],
                    )
                else:
                    # mid-stream: keep the gather-dependent dot off DVE
                    nc.gpsimd.tensor_tensor(
                        out=prodB[:], in0=rowU[:], in1=voB[:], op=OP.mult
                    )
                    nc.scalar.activation(
                        out=prodB[:], in_=prodB[:], func=ACTF.Copy,
                        accum_out=dall[:, t:t + 1],
                    )

            # c_out rides the idle SP queue (off the critical tail); d_out
            # on ACT (one DVE->ACT hop after the fused T4 dot)
            nc.sync.dma_start(out=c_out[:], in_=call[:])
            nc.scalar.dma_start(out=d_out[:], in_=dall[:])

    _split_multi_waits(nc)
    mybir.codegen_inst_isa_subclasses(nc)
    return nc


def _consts():
    p = np.arange(128)
    qbaseM = (MARK + (p % 4) * QW).astype(np.float32).reshape(128, 1)
    wc2 = np.tile(np.arange(2, dtype=np.float32), (128, 1))
    wc8 = np.tile(np.arange(8, dtype=np.float32), (128, 1))
    c8 = np.arange(8)
    c28 = np.concatenate([np.arange(3), np.repeat(np.arange(3, 8), 5), [7]])
    wc8b = np.concatenate(
        [np.tile(((c8 >> b) & 1).astype(np.float32), (128, 1)) for b in range(3)],
        axis=1)
    wc28b = np.concatenate(
        [np.tile(((c28 >> b) & 1).astype(np.float32), (128, 1)) for b in range(3)],
        axis=1)
    foldq = np.zeros((128, 32), np.float32)
    foldq[p, p // 4] = 1.0
    cc = np.concatenate([qbaseM, wc2, wc8b, wc28b, foldq], axis=1)
    # reps[t, b, p] = 1 iff partition p of tile t holds a row of batch b
    bmap = np.empty((4, 128), np.int64)
    bmap[0] = p // CTX                                   # vi rows 0..127
    bmap[1, :64] = (128 + p[:64]) // CTX                 # vi rows 128..191
    bmap[1, 64:] = (p[64:] - 64) // K                    # ng rows 0..63
    bmap[2] = (64 + p) // K                              # ng rows 64..191
    bmap[3] = (192 + p) // K                             # ng rows 192..319
    reps = np.zeros((4, 32, 128), np.float32)
    for t in range(4):
        reps[t, bmap[t], p] = 1.0
    reps = reps.transpose(1, 0, 2).reshape(32, 4 * 128)
    return cc, reps


_CACHE = {}


def kernel(vo, vi, neg_samples, V, U):
    if "nc" not in _CACHE:
        _CACHE["nc"] = _build()
        _CACHE["consts"] = _consts()
    nc = _CACHE["nc"]
    cc, reps = _CACHE["consts"]

    vo = np.ascontiguousarray(vo, dtype=np.float32)
    vi = np.ascontiguousarray(vi, dtype=np.float32)
    neg = np.ascontiguousarray(neg_samples, dtype=np.float32)
    V = np.ascontiguousarray(V, dtype=np.float32)
    U = np.ascontiguousarray(U, dtype=np.float32)

    in_maps = []
    for c in range(NCORES):
        sl = slice(c * BPC, (c + 1) * BPC)
        in_maps.append({
            "vo": vo[sl],
            "vi": vi[sl].reshape(NV, VOC),
            "ng": neg[sl].reshape(NN, VOC),
            "V": V,
            "U": U,
            "cc": cc, "reps": reps,
        })

    res = run_bass_kernel_spmd(nc, in_maps, list(range(NCORES)))
    obs = []
    for r in res.results:
        d = r["dout"]                                  # [128, 4]
        cc = r["cout"]                                 # [128, 4]
        d_vi = np.concatenate([d[:, 0], d[:64, 1]]).reshape(BPC, CTX)
        c_vi = np.concatenate([cc[:, 0], cc[:64, 1]]).reshape(BPC, CTX)
        d_ng = np.concatenate([d[64:, 1], d[:, 2], d[:, 3]]).reshape(BPC, K)
        lp = (d_vi * c_vi).sum(axis=1)
        ms = c_vi.sum(axis=1)
        x = lp / ms
        left = -np.log1p(np.exp(-x))
        right = (-np.log1p(np.exp(d_ng))).sum(axis=1)
        obs.append(-(left + right))
    ob = np.concatenate(obs)
    return np.float32(ob.mean(dtype=np.float64))


# revision 4
# speedup vs baseline: 1.1391x; 1.0004x over previous
"""CBOW negative-sampling loss kernel for 8 Trainium2 NeuronCores.

The reference computes one-hot @ table matmuls (embedding lookups in
disguise) followed by a tiny log-sigmoid loss.  Device-side algorithm
(v2: single fused DVE pass per byte, no DRAM scratch, no host iota):

Streaming extraction (DMA-bound, ~305us/core at the 360 GB/s limit):
  One-hot rows stream as 5 SBUF tiles of [128, *]:
    T0: the 32 vo rows split in 4 partition-quarters [128, 12500]
    T1..T4: the 192 vi + 320 neg rows as 4x [128, 50000]
  Chunks are [128, 6250]; T4's last 5 chunks are split into [128, 1250]
  pieces (last one 2x625) so the DVE tracks the DMA to the final byte.
  ONE fused DVE scalar_tensor_tensor per chunk multiplies by a
  Pool-generated iota tile (value MARK + j, MARK = 65536) and
  row-reduces into vals[:, col]; each one-hot row has <= one 1, so
  vals[p, col] = MARK + in-chunk-offset, exactly in fp32.
  Per tile the hit chunk c is recovered BIT-WISE (no HW divide): for
  each bit b, a weighted column reduce with weights bit_b(chunk(col))
  gives S2_b; bit_b = S2_b >= MARK.  Then
    off = max(sum_b 6250*2^b*bit_b + (S1 - MARK), 0),   cnt = S1 >= MARK
  all on DVE (no DMA deps, so the in-order DVE queue never stalls).
  For T0 a [128]->[32] one-hot fold matmul sums the 4 quarter
  contributions cnt*(MARK + 12500 q) + off into the global vo index.

Gathers + dots (overlapped with streaming of later tiles):
  V rows for vo are gathered once [32, 300] and replicated to each
  tile's partition order by a one-hot [32->128] matmul into PSUM (ACT
  copies PSUM->SBUF).  U rows are gathered per tile with single-offset
  indirect DMA driven straight from the extracted SBUF indices.  The
  per-row dot d = U_row . V_vo_row runs as Pool multiply + ACT
  accumulate mid-stream (never on the DVE queue, which would stall
  behind the gather), and as one fused DVE op for the final tile.

Host: batch-shard across 8 cores, log-sigmoid loss terms + mean of the
256 per-batch terms (same split as the v1 baseline).

Engine/ISA notes (hardware-verified): tensor_tensor_reduce (bass_isa
extended ISA) compiles but faults at runtime here - scalar_tensor_tensor
(core BIR, is_scalar_tensor_tensor=True) is the fused multiply+reduce
that actually runs.  Pool supports tensor_tensor {mult,add,sub} and
tensor_scalar {mult,sub,max,is_ge} only (no divide anywhere).
tensor_scalar with accum_out faults at runtime.
"""
import numpy as np

import concourse.bass as bass
import concourse.mybir as mybir
from concourse.tile import TileContext
from concourse.bass_utils import run_bass_kernel_spmd

VOC = 50000
EMB = 300
B = 256
CTX = 6
K = 10
NCORES = 8
BPC = B // NCORES                    # 32 batch rows per core
NV = BPC * CTX                       # 192 vi rows per core
NN = BPC * K                         # 320 neg rows per core
CH = 6250                            # free-dim chunk width
NCH = VOC // CH                      # 8 chunks per full 50000 row
QW = VOC // 4                        # 12500 per vo partition-quarter
MARK = 65536.0                       # presence marker (> max idx, power of 2)

F32 = mybir.dt.float32
I32 = mybir.dt.int32


def _split_multi_waits(nc):
    """This env's walrus accepts only ONE sync wait per instruction.
    Hoist extra waits into single-wait NoOps right before the owner."""
    cnt = 0
    for fn in nc.m.functions:
        for blk in fn.blocks:
            insts = list(blk.instructions)
            if not any(
                i.sync_info and i.sync_info.on_wait and len(i.sync_info.on_wait) > 1
                for i in insts
            ):
                continue
            new = []
            for inst in insts:
                si = inst.sync_info
                if si and si.on_wait and len(si.on_wait) > 1:
                    waits = list(si.on_wait)
                    for w in waits[:-1]:
                        cnt += 1
                        nop = mybir.InstNoOp(
                            name=f"mwsplit-{cnt}", engine=inst.engine, ins=[], outs=[]
                        )
                        nop.sync_info = mybir.SyncInfo(on_wait=[w], on_update=[])
                        new.append(nop)
                    inst.sync_info = mybir.SyncInfo(
                        on_wait=[waits[-1]], on_update=list(si.on_update or [])
                    )
                new.append(inst)
            blk.instructions = new
    return cnt


def _build():
    nc = bass.Bass(enable_partition_id=False)

    vo = nc.declare_dram_parameter("vo", [BPC, VOC], F32, isOutput=False)
    vi = nc.declare_dram_parameter("vi", [NV, VOC], F32, isOutput=False)
    ng = nc.declare_dram_parameter("ng", [NN, VOC], F32, isOutput=False)
    V = nc.declare_dram_parameter("V", [VOC, EMB], F32, isOutput=False)
    U = nc.declare_dram_parameter("U", [VOC, EMB], F32, isOutput=False)
    # merged consts: col 0 qbaseM | 1:3 wc2-bit0 | 3:27 wc8 bits | 27:114
    # wc29 bits | 114:146 foldq | 146:149 bit weights CH*2^b
    cc = nc.declare_dram_parameter("cc", [128, 149], F32, isOutput=False)
    reps = nc.declare_dram_parameter("reps", [32, 4 * 128], F32, isOutput=False)
    d_out = nc.declare_dram_parameter("dout", [128, 4], F32, isOutput=True)
    c_out = nc.declare_dram_parameter("cout", [128, 4], F32, isOutput=True)

    vo_q = vo.rearrange("r (q f) -> (r q) f", q=4)     # [128, 12500]
    # big tiles: list of (dram slice, partition range) DMAs per tile
    big = [
        [(vi[0:128, :], 0, 128)],
        [(vi[128:NV, :], 0, 64), (ng[0:64, :], 64, 128)],
        [(ng[64:192, :], 0, 128)],
        [(ng[192:NN, :], 0, 128)],
    ]

    AX = mybir.AxisListType.X
    OP = mybir.AluOpType
    ACTF = mybir.ActivationFunctionType

    with TileContext(nc) as tc:
        with (
            tc.tile_pool(name="const", bufs=1) as cpool,
            tc.tile_pool(name="data", bufs=3) as dpool,
            tc.tile_pool(name="prod", bufs=1) as ppool,
            tc.tile_pool(name="pieces", bufs=12) as qpool,
            tc.tile_pool(name="pieces2", bufs=2) as q2pool,
            tc.tile_pool(name="small", bufs=2) as spool,
            tc.tile_pool(name="keep", bufs=1) as kpool,
            tc.tile_pool(name="gath", bufs=2) as gpool,
            tc.tile_pool(name="psum", bufs=2, space="PSUM") as psum_pool,
        ):
            # on-device iota (value MARK + j, exact in fp32); consts go on
            # the ACT DMA queue so SP can start the big streaming DMAs at
            # once (keeps the serial DMA device fed from t=0)
            iota_t = cpool.tile([128, CH], F32, tag="iota")
            nc.gpsimd.iota(
                out=iota_t[:], pattern=[[1, CH]], base=int(MARK),
                channel_multiplier=0, allow_small_or_imprecise_dtypes=True,
            )
            dall = kpool.tile([128, 4], F32, tag="dall")
            call = kpool.tile([128, 4], F32, tag="call")

            def stream_tile(srcs, nch, vals, split_from=None):
                """DMA [128, CH] chunks + fused multiply-reduce each into
                vals[:, col].  Chunks >= split_from are 5 piecewise
                [128, 1250] DMAs+reduces (one vals column each, weight
                handled by the wc table) so DVE tracks the DMA closely and
                the kernel-tail dependency chain stays short."""
                if split_from is None:
                    split_from = nch
                col = 0
                for c in range(split_from):
                    chunk = dpool.tile([128, CH], F32, tag="chunk")
                    for src, p0, p1 in srcs:
                        nc.sync.dma_start(
                            out=chunk[p0:p1, :], in_=src[:, c * CH:(c + 1) * CH]
                        )
                    prod = ppool.tile([128, CH], F32, tag="prod")
                    nc.vector.scalar_tensor_tensor(
                        out=prod[:], in0=chunk[:], scalar=1.0, in1=iota_t[:],
                        op0=OP.mult, op1=OP.mult,
                        accum_out=vals[:, col:col + 1],
                    )
                    col += 1
                W = CH // 5
                subs = []
                for c in range(split_from, nch):
                    for h in range(5):
                        if c == nch - 1 and h == 4:
                            subs.extend([(c, h * W, W // 2), (c, h * W + W // 2, W // 2)])
                        else:
                            subs.append((c, h * W, W))
                for c, j0, w in subs:
                    pp = qpool if w == W else q2pool
                    piece = pp.tile([128, w], F32, tag=f"piece{w}")
                    for src, p0, p1 in srcs:
                        nc.sync.dma_start(
                            out=piece[p0:p1, :],
                            in_=src[:, c * CH + j0:c * CH + j0 + w],
                        )
                    prodp = ppool.tile([128, w], F32, tag=f"prodp{w}")
                    nc.vector.scalar_tensor_tensor(
                        out=prodp[:], in0=piece[:], scalar=1.0,
                        in1=iota_t[:, j0:j0 + w],
                        op0=OP.mult, op1=OP.mult,
                        accum_out=vals[:, col:col + 1],
                    )
                    col += 1

            def extract(vals, wcbits, ncol, cnt_out, off_out, tag):
                """cnt = (row had a 1); off = CH*c_hit + j_hit (0 if none).
                The hit-chunk index c is recovered bit-by-bit: one weighted
                column reduce per bit of c (weight = that bit of each
                column's chunk index), then is_ge(MARK) -> bit, scaled by
                CH*2^b and summed.  No division needed anywhere.  Heavy
                reduces on DVE (no DMA deps); scalar chain on Pool."""
                S1 = spool.tile([128, 1], F32, tag=f"S1{tag}")
                nc.vector.tensor_reduce(out=S1[:], in_=vals[:], axis=AX,
                                        op=OP.add)
                jp = spool.tile([128, 1], F32, tag=f"jp{tag}")
                nc.vector.tensor_scalar(
                    out=jp[:], in0=S1[:], scalar1=MARK, scalar2=None, op0=OP.subtract
                )
                nbits = len(wcbits)
                bbs = spool.tile([128, nbits], F32, tag=f"bbs{tag}")
                for b, wc in enumerate(wcbits):
                    S2 = spool.tile([128, 1], F32, tag=f"S2{tag}{b}")
                    junk = spool.tile([128, ncol], F32, tag=f"jk{ncol}")
                    nc.vector.scalar_tensor_tensor(
                        out=junk[:], in0=vals[:], scalar=1.0, in1=wc,
                        op0=OP.mult, op1=OP.mult, accum_out=S2[:],
                    )
                    nc.vector.tensor_scalar(
                        out=bbs[:, b:b + 1], in0=S2[:], scalar1=MARK,
                        scalar2=None, op0=OP.is_ge,
                    )
                nc.vector.tensor_scalar(
                    out=cnt_out, in0=S1[:], scalar1=MARK, scalar2=None, op0=OP.is_ge
                )
                # acc = sum_b bit_b * CH*2^b in ONE fused weighted reduce
                junkb = spool.tile([128, nbits], F32, tag=f"jb{tag}")
                acc = spool.tile([128, 1], F32, tag=f"acc{tag}")
                nc.vector.scalar_tensor_tensor(
                    out=junkb[:], in0=bbs[:], scalar=1.0,
                    in1=wpow_t[:, :nbits],
                    op0=OP.mult, op1=OP.mult, accum_out=acc[:],
                )
                # off = max(acc + (S1 - MARK), 0): a no-hit row gives
                # acc=0, S1=0 -> clamps to 0; keeps cnt off this path
                aj = spool.tile([128, 1], F32, tag=f"aj{tag}")
                nc.vector.tensor_tensor(out=aj[:], in0=acc[:], in1=jp[:], op=OP.add)
                nc.vector.tensor_scalar(
                    out=off_out, in0=aj[:], scalar1=0.0, scalar2=None, op0=OP.max
                )

            # ---------------- T0: vo quarters ----------------
            vals0 = spool.tile([128, 2], F32, tag="vals0")
            stream_tile([(vo_q, 0, 128)], 2, vals0)

            # small constants: two merged DMAs on the ACT queue (after the
            # first big chunks, one HWDGE generation each)
            cc_t = cpool.tile([128, 149], F32, tag="cc")
            nc.scalar.dma_start(out=cc_t[:], in_=cc[:])
            reps_all = cpool.tile([32, 4 * 128], F32, tag="reps")
            nc.scalar.dma_start(out=reps_all[:], in_=reps[:])
            qbaseM_t = cc_t[:, 0:1]
            wc2_bits = [cc_t[:, 1:3]]
            wc8_bits = [cc_t[:, 3 + 8 * b:11 + 8 * b] for b in range(3)]
            wc28_bits = [cc_t[:, 27 + 29 * b:56 + 29 * b] for b in range(3)]
            foldq_t = cc_t[:, 114:146]
            wpow_t = cc_t[:, 146:149]
            reps_t = [reps_all[:, 128 * t:128 * (t + 1)] for t in range(4)]

            cnt0 = spool.tile([128, 1], F32, tag="cnt0")
            off0 = spool.tile([128, 1], F32, tag="off0")
            extract(vals0, wc2_bits, 2, cnt0[:], off0[:], "t0")
            # X = cnt*(MARK + 12500 q) + off ; fold quarters -> [32, 1]
            xq = spool.tile([128, 1], F32, tag="xq")
            nc.vector.tensor_tensor(out=xq[:], in0=cnt0[:], in1=qbaseM_t, op=OP.mult)
            x2 = spool.tile([128, 1], F32, tag="x2")
            nc.vector.tensor_tensor(out=x2[:], in0=xq[:], in1=off0[:], op=OP.add)
            pfold = psum_pool.tile([32, 1], F32, tag="p32")
            nc.tensor.matmul(out=pfold[:], lhsT=foldq_t, rhs=x2[:],
                             start=True, stop=True)
            idxvo = spool.tile([32, 1], F32, tag="idxvo")
            nc.vector.tensor_scalar(
                out=idxvo[:], in0=pfold[:], scalar1=MARK, scalar2=None,
                op0=OP.subtract,
            )
            ofs_v = spool.tile([32, 1], I32, tag="ofsv")
            nc.vector.tensor_copy(out=ofs_v[:], in_=idxvo[:])
            voV = cpool.tile([32, EMB], F32, tag="voV")
            nc.gpsimd.indirect_dma_start(
                out=voV[:], out_offset=None, in_=V[:],
                in_offset=bass.IndirectOffsetOnAxis(ap=ofs_v[:], axis=0),
            )

            # ---------------- T1..T4: vi + neg rows ----------------
            # The per-tile dot runs as Pool multiply + ACT accumulate —
            # neither touches the DVE streaming queue, so a gather that
            # lands late can only block Pool (whose next deadline is a
            # full tile away).
            for t in range(4):
                last = t == 3
                ncol = 3 + 26 if last else NCH
                vals = spool.tile([128, ncol], F32, tag=f"vals{ncol}")
                stream_tile(big[t], NCH, vals, split_from=3 if last else None)
                off = spool.tile([128, 1], F32, tag="off")
                extract(vals, wc28_bits if last else wc8_bits, ncol,
                        call[:, t:t + 1], off[:], "tb")
                ofs_u = spool.tile([128, 1], I32, tag="ofsu")
                nc.vector.tensor_copy(out=ofs_u[:], in_=off[:])
                rowU = gpool.tile([128, EMB], F32, tag="rowU")
                nc.gpsimd.indirect_dma_start(
                    out=rowU[:], out_offset=None, in_=U[:],
                    in_offset=bass.IndirectOffsetOnAxis(ap=ofs_u[:], axis=0),
                )
                pB = psum_pool.tile([128, EMB], F32, tag="pB")
                nc.tensor.matmul(out=pB[:], lhsT=reps_t[t], rhs=voV[:],
                                 start=True, stop=True)
                voB = gpool.tile([128, EMB], F32, tag="voB")
                nc.scalar.activation(out=voB[:], in_=pB[:], func=ACTF.Copy)
                prodB = gpool.tile([128, EMB], F32, tag="prodB")
                if last:
                    # tail: one fused DVE op (DVE is idle by now)
                    nc.vector.scalar_tensor_tensor(
                        out=prodB[:], in0=rowU[:], scalar=1.0, in1=voB[:],
                        op0=OP.mult, op1=OP.mult, accum_out=dall[:, t:t + 1],
                    )
                else:
                    # mid-stream: keep the gather-dependent dot off DVE
                    nc.gpsimd.tensor_tensor(
                        out=prodB[:], in0=rowU[:], in1=voB[:], op=OP.mult
                    )
                    nc.scalar.activation(
                        out=prodB[:], in_=prodB[:], func=ACTF.Copy,
                        accum_out=dall[:, t:t + 1],
                    )

            # c_out rides the idle SP queue (off the critical tail); d_out
            # on ACT (one DVE->ACT hop after the fused T4 dot)
            nc.sync.dma_start(out=c_out[:], in_=call[:])
            nc.scalar.dma_start(out=d_out[:], in_=dall[:])

    _split_multi_waits(nc)
    mybir.codegen_inst_isa_subclasses(nc)
    return nc


def _consts():
    p = np.arange(128)
    qbaseM = (MARK + (p % 4) * QW).astype(np.float32).reshape(128, 1)
    wc2 = np.tile(np.arange(2, dtype=np.float32), (128, 1))
    wc8 = np.tile(np.arange(8, dtype=np.float32), (128, 1))
    c8 = np.arange(8)
    c28 = np.concatenate([np.arange(3), np.repeat(np.arange(3, 8), 5), [7]])
    wc8b = np.concatenate(
        [np.tile(((c8 >> b) & 1).astype(np.float32), (128, 1)) for b in range(3)],
        axis=1)
    wc28b = np.concatenate(
        [np.tile(((c28 >> b) & 1).astype(np.float32), (128, 1)) for b in range(3)],
        axis=1)
    foldq = np.zeros((128, 32), np.float32)
    foldq[p, p // 4] = 1.0
    wpow = np.tile(np.array([6250.0, 12500.0, 25000.0],
                            dtype=np.float32), (128, 1))
    cc = np.concatenate([qbaseM, wc2, wc8b, wc28b, foldq, wpow], axis=1)
    # reps[t, b, p] = 1 iff partition p of tile t holds a row of batch b
    bmap = np.empty((4, 128), np.int64)
    bmap[0] = p // CTX                                   # vi rows 0..127
    bmap[1, :64] = (128 + p[:64]) // CTX                 # vi rows 128..191
    bmap[1, 64:] = (p[64:] - 64) // K                    # ng rows 0..63
    bmap[2] = (64 + p) // K                              # ng rows 64..191
    bmap[3] = (192 + p) // K                             # ng rows 192..319
    reps = np.zeros((4, 32, 128), np.float32)
    for t in range(4):
        reps[t, bmap[t], p] = 1.0
    reps = reps.transpose(1, 0, 2).reshape(32, 4 * 128)
    return cc, reps


_CACHE = {}


def kernel(vo, vi, neg_samples, V, U):
    if "nc" not in _CACHE:
        _CACHE["nc"] = _build()
        _CACHE["consts"] = _consts()
    nc = _CACHE["nc"]
    cc, reps = _CACHE["consts"]

    vo = np.ascontiguousarray(vo, dtype=np.float32)
    vi = np.ascontiguousarray(vi, dtype=np.float32)
    neg = np.ascontiguousarray(neg_samples, dtype=np.float32)
    V = np.ascontiguousarray(V, dtype=np.float32)
    U = np.ascontiguousarray(U, dtype=np.float32)

    in_maps = []
    for c in range(NCORES):
        sl = slice(c * BPC, (c + 1) * BPC)
        in_maps.append({
            "vo": vo[sl],
            "vi": vi[sl].reshape(NV, VOC),
            "ng": neg[sl].reshape(NN, VOC),
            "V": V,
            "U": U,
            "cc": cc, "reps": reps,
        })

    res = run_bass_kernel_spmd(nc, in_maps, list(range(NCORES)))
    obs = []
    for r in res.results:
        d = r["dout"]                                  # [128, 4]
        cc = r["cout"]                                 # [128, 4]
        d_vi = np.concatenate([d[:, 0], d[:64, 1]]).reshape(BPC, CTX)
        c_vi = np.concatenate([cc[:, 0], cc[:64, 1]]).reshape(BPC, CTX)
        d_ng = np.concatenate([d[64:, 1], d[:, 2], d[:, 3]]).reshape(BPC, K)
        lp = (d_vi * c_vi).sum(axis=1)
        ms = c_vi.sum(axis=1)
        x = lp / ms
        left = -np.log1p(np.exp(-x))
        right = (-np.log1p(np.exp(d_ng))).sum(axis=1)
        obs.append(-(left + right))
    ob = np.concatenate(obs)
    return np.float32(ob.mean(dtype=np.float64))


# revision 5
# speedup vs baseline: 1.1398x; 1.0007x over previous
"""CBOW negative-sampling loss kernel for 8 Trainium2 NeuronCores.

The reference computes one-hot @ table matmuls (embedding lookups in
disguise) followed by a tiny log-sigmoid loss.  Device-side algorithm
(v2: single fused DVE pass per byte, no DRAM scratch, no host iota):

Streaming extraction (DMA-bound, ~305us/core at the 360 GB/s limit):
  One-hot rows stream as 5 SBUF tiles of [128, *]:
    T0: the 32 vo rows split in 4 partition-quarters [128, 12500]
    T1..T4: the 192 vi + 320 neg rows as 4x [128, 50000]
  Chunks are [128, 6250]; T4's last 5 chunks are split into [128, 1250]
  pieces (last one 2x625) so the DVE tracks the DMA to the final byte.
  ONE fused DVE scalar_tensor_tensor per chunk multiplies by a
  Pool-generated iota tile (value MARK + j, MARK = 65536) and
  row-reduces into vals[:, col]; each one-hot row has <= one 1, so
  vals[p, col] = MARK + in-chunk-offset, exactly in fp32.
  Per tile the hit chunk c is recovered BIT-WISE (no HW divide): for
  each bit b, a weighted column reduce with weights bit_b(chunk(col))
  gives S2_b; bit_b = S2_b >= MARK.  Then
    off = max(sum_b 6250*2^b*bit_b + (S1 - MARK), 0),   cnt = S1 >= MARK
  all on DVE (no DMA deps, so the in-order DVE queue never stalls).
  For T0 a [128]->[32] one-hot fold matmul sums the 4 quarter
  contributions cnt*(MARK + 12500 q) + off into the global vo index.

Gathers + dots (overlapped with streaming of later tiles):
  V rows for vo are gathered once [32, 300] and replicated to each
  tile's partition order by a one-hot [32->128] matmul into PSUM (ACT
  copies PSUM->SBUF).  U rows are gathered per tile with single-offset
  indirect DMA driven straight from the extracted SBUF indices.  The
  per-row dot d = U_row . V_vo_row runs as Pool multiply + ACT
  accumulate mid-stream (never on the DVE queue, which would stall
  behind the gather), and as one fused DVE op for the final tile.

Host: batch-shard across 8 cores, log-sigmoid loss terms + mean of the
256 per-batch terms (same split as the v1 baseline).

Engine/ISA notes (hardware-verified): tensor_tensor_reduce (bass_isa
extended ISA) compiles but faults at runtime here - scalar_tensor_tensor
(core BIR, is_scalar_tensor_tensor=True) is the fused multiply+reduce
that actually runs.  Pool supports tensor_tensor {mult,add,sub} and
tensor_scalar {mult,sub,max,is_ge} only (no divide anywhere).
tensor_scalar with accum_out faults at runtime.
"""
import numpy as np

import concourse.bass as bass
import concourse.mybir as mybir
from concourse.tile import TileContext
from concourse.bass_utils import run_bass_kernel_spmd

VOC = 50000
EMB = 300
B = 256
CTX = 6
K = 10
NCORES = 8
BPC = B // NCORES                    # 32 batch rows per core
NV = BPC * CTX                       # 192 vi rows per core
NN = BPC * K                         # 320 neg rows per core
CH = 6250                            # free-dim chunk width
NCH = VOC // CH                      # 8 chunks per full 50000 row
QW = VOC // 4                        # 12500 per vo partition-quarter
MARK = 65536.0                       # presence marker (> max idx, power of 2)

F32 = mybir.dt.float32
I32 = mybir.dt.int32


def _split_multi_waits(nc):
    """This env's walrus accepts only ONE sync wait per instruction.
    Hoist extra waits into single-wait NoOps right before the owner."""
    cnt = 0
    for fn in nc.m.functions:
        for blk in fn.blocks:
            insts = list(blk.instructions)
            if not any(
                i.sync_info and i.sync_info.on_wait and len(i.sync_info.on_wait) > 1
                for i in insts
            ):
                continue
            new = []
            for inst in insts:
                si = inst.sync_info
                if si and si.on_wait and len(si.on_wait) > 1:
                    waits = list(si.on_wait)
                    for w in waits[:-1]:
                        cnt += 1
                        nop = mybir.InstNoOp(
                            name=f"mwsplit-{cnt}", engine=inst.engine, ins=[], outs=[]
                        )
                        nop.sync_info = mybir.SyncInfo(on_wait=[w], on_update=[])
                        new.append(nop)
                    inst.sync_info = mybir.SyncInfo(
                        on_wait=[waits[-1]], on_update=list(si.on_update or [])
                    )
                new.append(inst)
            blk.instructions = new
    return cnt


def _build():
    nc = bass.Bass(enable_partition_id=False)

    vo = nc.declare_dram_parameter("vo", [BPC, VOC], F32, isOutput=False)
    vi = nc.declare_dram_parameter("vi", [NV, VOC], F32, isOutput=False)
    ng = nc.declare_dram_parameter("ng", [NN, VOC], F32, isOutput=False)
    V = nc.declare_dram_parameter("V", [VOC, EMB], F32, isOutput=False)
    U = nc.declare_dram_parameter("U", [VOC, EMB], F32, isOutput=False)
    # merged consts: col 0 qbaseM | 1:3 wc2-bit0 | 3:27 wc8 bits | 27:117
    # wc30 bits | 117:149 foldq | 149:152 bit weights CH*2^b
    cc = nc.declare_dram_parameter("cc", [128, 152], F32, isOutput=False)
    reps = nc.declare_dram_parameter("reps", [32, 4 * 128], F32, isOutput=False)
    d_out = nc.declare_dram_parameter("dout", [128, 4], F32, isOutput=True)
    c_out = nc.declare_dram_parameter("cout", [128, 4], F32, isOutput=True)

    vo_q = vo.rearrange("r (q f) -> (r q) f", q=4)     # [128, 12500]
    # big tiles: list of (dram slice, partition range) DMAs per tile
    big = [
        [(vi[0:128, :], 0, 128)],
        [(vi[128:NV, :], 0, 64), (ng[0:64, :], 64, 128)],
        [(ng[64:192, :], 0, 128)],
        [(ng[192:NN, :], 0, 128)],
    ]

    AX = mybir.AxisListType.X
    OP = mybir.AluOpType
    ACTF = mybir.ActivationFunctionType

    with TileContext(nc) as tc:
        with (
            tc.tile_pool(name="const", bufs=1) as cpool,
            tc.tile_pool(name="data", bufs=3) as dpool,
            tc.tile_pool(name="pieces", bufs=12) as qpool,
            tc.tile_pool(name="pieces2", bufs=2) as q2pool,
            tc.tile_pool(name="small", bufs=2) as spool,
            tc.tile_pool(name="keep", bufs=1) as kpool,
            tc.tile_pool(name="gath", bufs=2) as gpool,
            tc.tile_pool(name="psum", bufs=2, space="PSUM") as psum_pool,
        ):
            # on-device iota (value MARK + j, exact in fp32); consts go on
            # the ACT DMA queue so SP can start the big streaming DMAs at
            # once (keeps the serial DMA device fed from t=0)
            iota_t = cpool.tile([128, CH], F32, tag="iota")
            nc.gpsimd.iota(
                out=iota_t[:], pattern=[[1, CH]], base=int(MARK),
                channel_multiplier=0, allow_small_or_imprecise_dtypes=True,
            )
            dall = kpool.tile([128, 4], F32, tag="dall")
            call = kpool.tile([128, 4], F32, tag="call")

            def stream_tile(srcs, nch, vals, split_from=None):
                """DMA [128, CH] chunks + fused multiply-reduce each into
                vals[:, col].  Chunks >= split_from are 5 piecewise
                [128, 1250] DMAs+reduces (one vals column each, weight
                handled by the wc table) so DVE tracks the DMA closely and
                the kernel-tail dependency chain stays short."""
                if split_from is None:
                    split_from = nch
                col = 0
                for c in range(split_from):
                    chunk = dpool.tile([128, CH], F32, tag="chunk")
                    for src, p0, p1 in srcs:
                        nc.sync.dma_start(
                            out=chunk[p0:p1, :], in_=src[:, c * CH:(c + 1) * CH]
                        )
                    nc.vector.scalar_tensor_tensor(
                        out=chunk[:], in0=chunk[:], scalar=1.0, in1=iota_t[:],
                        op0=OP.mult, op1=OP.mult,
                        accum_out=vals[:, col:col + 1],
                    )
                    col += 1
                W = CH // 5
                subs = []
                for c in range(split_from, nch):
                    for h in range(5):
                        if c == nch - 1 and h == 4:
                            subs.extend([(c, h * W, 625), (c, h * W + 625, 313),
                                         (c, h * W + 938, 312)])
                        else:
                            subs.append((c, h * W, W))
                for c, j0, w in subs:
                    pp = qpool if w == W else q2pool
                    piece = pp.tile([128, w], F32, tag=f"piece{w}")
                    for src, p0, p1 in srcs:
                        nc.sync.dma_start(
                            out=piece[p0:p1, :],
                            in_=src[:, c * CH + j0:c * CH + j0 + w],
                        )
                    nc.vector.scalar_tensor_tensor(
                        out=piece[:], in0=piece[:], scalar=1.0,
                        in1=iota_t[:, j0:j0 + w],
                        op0=OP.mult, op1=OP.mult,
                        accum_out=vals[:, col:col + 1],
                    )
                    col += 1

            def extract(vals, wcbits, ncol, cnt_out, off_out, tag):
                """cnt = (row had a 1); off = CH*c_hit + j_hit (0 if none).
                The hit-chunk index c is recovered bit-by-bit: one weighted
                column reduce per bit of c (weight = that bit of each
                column's chunk index), then is_ge(MARK) -> bit, scaled by
                CH*2^b and summed.  No division needed anywhere.  Heavy
                reduces on DVE (no DMA deps); scalar chain on Pool."""
                S1 = spool.tile([128, 1], F32, tag=f"S1{tag}")
                nc.vector.tensor_reduce(out=S1[:], in_=vals[:], axis=AX,
                                        op=OP.add)
                nbits = len(wcbits)
                bbs = spool.tile([128, nbits], F32, tag=f"bbs{tag}")
                for b, wc in enumerate(wcbits):
                    S2 = spool.tile([128, 1], F32, tag=f"S2{tag}{b}")
                    junk = spool.tile([128, ncol], F32, tag=f"jk{ncol}")
                    nc.vector.scalar_tensor_tensor(
                        out=junk[:], in0=vals[:], scalar=1.0, in1=wc,
                        op0=OP.mult, op1=OP.mult, accum_out=S2[:],
                    )
                    nc.vector.tensor_scalar(
                        out=bbs[:, b:b + 1], in0=S2[:], scalar1=MARK,
                        scalar2=None, op0=OP.is_ge,
                    )
                nc.vector.tensor_scalar(
                    out=cnt_out, in0=S1[:], scalar1=MARK, scalar2=None, op0=OP.is_ge
                )
                # acc = sum_b bit_b * CH*2^b in ONE fused weighted reduce
                junkb = spool.tile([128, nbits], F32, tag=f"jb{tag}")
                acc = spool.tile([128, 1], F32, tag=f"acc{tag}")
                nc.vector.scalar_tensor_tensor(
                    out=junkb[:], in0=bbs[:], scalar=1.0,
                    in1=wpow_t[:, :nbits],
                    op0=OP.mult, op1=OP.mult, accum_out=acc[:],
                )
                # off = max(acc - MARK + S1, 0): a no-hit row gives acc=0,
                # S1=0 -> clamps to 0; keeps cnt off this path.  The clamp
                # writes off_out directly (an i32 tile for the gather path).
                aj = spool.tile([128, 1], F32, tag=f"aj{tag}")
                nc.vector.scalar_tensor_tensor(
                    out=aj[:], in0=acc[:], scalar=-MARK, in1=S1[:],
                    op0=OP.add, op1=OP.add,
                )
                nc.vector.tensor_scalar(
                    out=off_out, in0=aj[:], scalar1=0.0, scalar2=None, op0=OP.max
                )

            # ---------------- T0: vo quarters ----------------
            vals0 = spool.tile([128, 2], F32, tag="vals0")
            stream_tile([(vo_q, 0, 128)], 2, vals0)

            # small constants: two merged DMAs on the ACT queue (after the
            # first big chunks, one HWDGE generation each)
            cc_t = cpool.tile([128, 152], F32, tag="cc")
            nc.scalar.dma_start(out=cc_t[:], in_=cc[:])
            reps_all = cpool.tile([32, 4 * 128], F32, tag="reps")
            nc.scalar.dma_start(out=reps_all[:], in_=reps[:])
            qbaseM_t = cc_t[:, 0:1]
            wc2_bits = [cc_t[:, 1:3]]
            wc8_bits = [cc_t[:, 3 + 8 * b:11 + 8 * b] for b in range(3)]
            wc28_bits = [cc_t[:, 27 + 30 * b:57 + 30 * b] for b in range(3)]
            foldq_t = cc_t[:, 117:149]
            wpow_t = cc_t[:, 149:152]
            reps_t = [reps_all[:, 128 * t:128 * (t + 1)] for t in range(4)]

            cnt0 = spool.tile([128, 1], F32, tag="cnt0")
            off0 = spool.tile([128, 1], F32, tag="off0")
            extract(vals0, wc2_bits, 2, cnt0[:], off0[:], "t0")
            # X = cnt*(MARK + 12500 q) + off ; fold quarters -> [32, 1]
            xq = spool.tile([128, 1], F32, tag="xq")
            nc.vector.tensor_tensor(out=xq[:], in0=cnt0[:], in1=qbaseM_t, op=OP.mult)
            x2 = spool.tile([128, 1], F32, tag="x2")
            nc.vector.tensor_tensor(out=x2[:], in0=xq[:], in1=off0[:], op=OP.add)
            pfold = psum_pool.tile([32, 1], F32, tag="p32")
            nc.tensor.matmul(out=pfold[:], lhsT=foldq_t, rhs=x2[:],
                             start=True, stop=True)
            ofs_v = spool.tile([32, 1], I32, tag="ofsv")
            nc.vector.tensor_scalar(
                out=ofs_v[:], in0=pfold[:], scalar1=MARK, scalar2=None,
                op0=OP.subtract,
            )
            voV = cpool.tile([32, EMB], F32, tag="voV")
            nc.gpsimd.indirect_dma_start(
                out=voV[:], out_offset=None, in_=V[:],
                in_offset=bass.IndirectOffsetOnAxis(ap=ofs_v[:], axis=0),
            )

            # ---------------- T1..T4: vi + neg rows ----------------
            # The per-tile dot runs as Pool multiply + ACT accumulate —
            # neither touches the DVE streaming queue, so a gather that
            # lands late can only block Pool (whose next deadline is a
            # full tile away).
            for t in range(4):
                last = t == 3
                ncol = 3 + 27 if last else NCH
                vals = spool.tile([128, ncol], F32, tag=f"vals{ncol}")
                stream_tile(big[t], NCH, vals, split_from=3 if last else None)
                ofs_u = spool.tile([128, 1], I32, tag="ofsu")
                extract(vals, wc28_bits if last else wc8_bits, ncol,
                        call[:, t:t + 1], ofs_u[:], "tb")
                rowU = gpool.tile([128, EMB], F32, tag="rowU")
                nc.gpsimd.indirect_dma_start(
                    out=rowU[:], out_offset=None, in_=U[:],
                    in_offset=bass.IndirectOffsetOnAxis(ap=ofs_u[:], axis=0),
                )
                pB = psum_pool.tile([128, EMB], F32, tag="pB")
                nc.tensor.matmul(out=pB[:], lhsT=reps_t[t], rhs=voV[:],
                                 start=True, stop=True)
                voB = gpool.tile([128, EMB], F32, tag="voB")
                nc.scalar.activation(out=voB[:], in_=pB[:], func=ACTF.Copy)
                prodB = gpool.tile([128, EMB], F32, tag="prodB")
                if last:
                    # tail: one fused DVE op (DVE is idle by now)
                    nc.vector.scalar_tensor_tensor(
                        out=prodB[:], in0=rowU[:], scalar=1.0, in1=voB[:],
                        op0=OP.mult, op1=OP.mult, accum_out=dall[:, t:t + 1],
                    )
                else:
                    # mid-stream: keep the gather-dependent dot off DVE
                    nc.gpsimd.tensor_tensor(
                        out=prodB[:], in0=rowU[:], in1=voB[:], op=OP.mult
                    )
                    nc.scalar.activation(
                        out=prodB[:], in_=prodB[:], func=ACTF.Copy,
                        accum_out=dall[:, t:t + 1],
                    )

            # c_out rides the idle SP queue (off the critical tail); d_out
            # on ACT (one DVE->ACT hop after the fused T4 dot)
            nc.sync.dma_start(out=c_out[:], in_=call[:])
            nc.scalar.dma_start(out=d_out[:], in_=dall[:])

    _split_multi_waits(nc)
    mybir.codegen_inst_isa_subclasses(nc)
    return nc


def _consts():
    p = np.arange(128)
    qbaseM = (MARK + (p % 4) * QW).astype(np.float32).reshape(128, 1)
    wc2 = np.tile(np.arange(2, dtype=np.float32), (128, 1))
    wc8 = np.tile(np.arange(8, dtype=np.float32), (128, 1))
    c8 = np.arange(8)
    c28 = np.concatenate([np.arange(3), np.repeat(np.arange(3, 8), 5), [7, 7]])
    wc8b = np.concatenate(
        [np.tile(((c8 >> b) & 1).astype(np.float32), (128, 1)) for b in range(3)],
        axis=1)
    wc28b = np.concatenate(
        [np.tile(((c28 >> b) & 1).astype(np.float32), (128, 1)) for b in range(3)],
        axis=1)
    foldq = np.zeros((128, 32), np.float32)
    foldq[p, p // 4] = 1.0
    wpow = np.tile(np.array([6250.0, 12500.0, 25000.0],
                            dtype=np.float32), (128, 1))
    cc = np.concatenate([qbaseM, wc2, wc8b, wc28b, foldq, wpow], axis=1)
    # reps[t, b, p] = 1 iff partition p of tile t holds a row of batch b
    bmap = np.empty((4, 128), np.int64)
    bmap[0] = p // CTX                                   # vi rows 0..127
    bmap[1, :64] = (128 + p[:64]) // CTX                 # vi rows 128..191
    bmap[1, 64:] = (p[64:] - 64) // K                    # ng rows 0..63
    bmap[2] = (64 + p) // K                              # ng rows 64..191
    bmap[3] = (192 + p) // K                             # ng rows 192..319
    reps = np.zeros((4, 32, 128), np.float32)
    for t in range(4):
        reps[t, bmap[t], p] = 1.0
    reps = reps.transpose(1, 0, 2).reshape(32, 4 * 128)
    return cc, reps


_CACHE = {}


def kernel(vo, vi, neg_samples, V, U):
    if "nc" not in _CACHE:
        _CACHE["nc"] = _build()
        _CACHE["consts"] = _consts()
    nc = _CACHE["nc"]
    cc, reps = _CACHE["consts"]

    vo = np.ascontiguousarray(vo, dtype=np.float32)
    vi = np.ascontiguousarray(vi, dtype=np.float32)
    neg = np.ascontiguousarray(neg_samples, dtype=np.float32)
    V = np.ascontiguousarray(V, dtype=np.float32)
    U = np.ascontiguousarray(U, dtype=np.float32)

    in_maps = []
    for c in range(NCORES):
        sl = slice(c * BPC, (c + 1) * BPC)
        in_maps.append({
            "vo": vo[sl],
            "vi": vi[sl].reshape(NV, VOC),
            "ng": neg[sl].reshape(NN, VOC),
            "V": V,
            "U": U,
            "cc": cc, "reps": reps,
        })

    res = run_bass_kernel_spmd(nc, in_maps, list(range(NCORES)))
    obs = []
    for r in res.results:
        d = r["dout"]                                  # [128, 4]
        cc = r["cout"]                                 # [128, 4]
        d_vi = np.concatenate([d[:, 0], d[:64, 1]]).reshape(BPC, CTX)
        c_vi = np.concatenate([cc[:, 0], cc[:64, 1]]).reshape(BPC, CTX)
        d_ng = np.concatenate([d[64:, 1], d[:, 2], d[:, 3]]).reshape(BPC, K)
        lp = (d_vi * c_vi).sum(axis=1)
        ms = c_vi.sum(axis=1)
        x = lp / ms
        left = -np.log1p(np.exp(-x))
        right = (-np.log1p(np.exp(d_ng))).sum(axis=1)
        obs.append(-(left + right))
    ob = np.concatenate(obs)
    return np.float32(ob.mean(dtype=np.float64))
